# revision 53
# baseline (speedup 1.0000x reference)
"""Trainium2 Bass kernel for nn_Block_30107720745811 (dense transformer block).

B=4, S=1024, H=1024, NH=16. 8 NeuronCores, zero-communication sharding:
core c computes batch b=c//2, query rows (c%2)*512:(c%2)*512+512.

All big GEMMs run as fp8e4m3 DoubleRow matmuls (0.5 cycles/row with a packed
256-deep contraction => 4x the fp32r PE rate). Activations live feature-major
[feature, token]; fp8 operands use the DoubleRow fold layout [128, 2, N]
(features 256c+128f+p at [p, f, n]). Q/K projections emit a d-folded layout
(head h=4a+b dim 32g+p at kf[32b+p, a, g, t]) via host-permuted weight
columns, so the per-head scores GEMM is also DoubleRow (K_p=32). The softmax
exp converts straight to fp8 folded tiles consumed by the att@V DoubleRow.
Residual-path tensors (xq, ca, h, ff) stay f32 for precision; the softmax
denominator rides att@V as a ones column of V.
"""
import numpy as np
import ml_dtypes
import concourse.bass as bass
import concourse.tile as tile
from concourse import mybir
from concourse import bass_utils
from concourse.alu_op_type import AluOpType as OP

AF = mybir.ActivationFunctionType
F32 = mybir.dt.float32
F32R = mybir.dt.float32r
F8 = mybir.dt.float8e4
DR = mybir.MatmulPerfMode.DoubleRow
FP8 = ml_dtypes.float8_e4m3

B, S, H, NH = 4, 1024, 1024, 16
D = H // NH          # 64
P = 128
T = 512              # query tokens per core
KC = H // P          # 8 feature chunks
C2 = 4               # 256-wide fold chunks
NS = 256             # V-proj output slice (4 heads)
VP = 80              # padded per-head vt row (D+1 used, 16B-aligned pitch)
INF = 1e10
EPS = 1e-5
SCALE = 8.0

# vec tensor column map
C_SBQ, C_SBK, C_SBO = 0, 8, 16
C_CBQ, C_CBK, C_CBO = 24, 32, 40
C_SAB, C_CAB = 48, 56
C_G, C_B = 64, 72
C_B1, C_B2, C_EPS = 80, 112, 120
C_CQS = 121
NVEC = 129

MAX_WAITS = 1


def _legalize_waits(nc, max_waits=MAX_WAITS):
    """Split >max_waits semaphore waits into preceding same-engine NOPs
    (this walrus build allows only one sync wait per instruction)."""
    n_split = 0
    for f in nc.m.functions:
        for blk in f.blocks:
            out = []
            for ins in blk.instructions:
                si = getattr(ins, "sync_info", None)
                if si is not None and si.on_wait and len(si.on_wait) > max_waits:
                    waits = list(si.on_wait)
                    extra, keep = waits[:-max_waits], waits[-max_waits:]
                    for j in range(0, len(extra), max_waits):
                        out.append(mybir.InstNoOp(
                            name=f"{ins.name}-lw{j}",
                            engine=ins.engine,
                            sync_info=mybir.SyncInfo(
                                on_wait=extra[j:j + max_waits], on_update=[]),
                            bass_nofuse=True,
                        ))
                    ins.sync_info = mybir.SyncInfo(
                        on_wait=keep, on_update=list(si.on_update))
                    n_split += 1
                out.append(ins)
            blk.instructions = out
    return n_split


def _build(dbg=False, masked=False, gbtriv=True):
    nc = bass.Bass("TRN2", target_bir_lowering=False, debug=False,
                   dynamic_dma_scratch_size=8192)

    def din(name, shape, dt=F32):
        return nc.dram_tensor(name, shape, dt, kind="ExternalInput").ap()

    xk_d = din("xkT", [H, S])            # self hidden, transposed
    xc_d = din("xcT", [H, S])            # cross hidden, transposed
    xq_d = din("xqT", [H, T])            # query cols of self hidden
    w8 = {}
    for p_ in ("s", "c"):
        w8[p_ + "qf"] = din(p_ + "WqF", [P, C2, 4, 2, 2, P], F8)
        w8[p_ + "kf"] = din(p_ + "WkF", [P, C2, 4, 2, 2, P], F8)
        w8[p_ + "v"] = din(p_ + "Wv8", [P, C2, 2, H], F8)
        w8[p_ + "o"] = din(p_ + "Wo8", [P, C2, 2, KC, P], F8)
    w18_d = din("W18", [P, 4, 2, C2, 2, 8, P], F8)  # m-blk, hi/lo outermost
    w28_d = din("W28", [P, KC, 2, 16, 2, P], F8)    # out-chunk m, hi/lo
    vec_d = din("vec", [P, NVEC], F32)
    ones2_d = din("ones2", [P, P], F32R)
    out_d = nc.dram_tensor("out", [H, T], F32, kind="ExternalOutput").ap()
    dbg_d = {}
    if dbg:
        for n, shape, dt in [("d_kf", [P, C2, 2, S], F8),
                             ("d_qf", [P, C2, 2, T], F8),
                             ("d_vt", [P, C2, 2, NH, VP], F8),
                             ("d_at", [P, C2, 2, T], F8),
                             ("d_sa", [P, C2, 2, T], F8),
                             ("d_snn", [P, C2, 2, T], F8),
                             ("d_h", [H, T], F32),
                             ("d_u", [P, 16, 2, T], F8)]:
            dbg_d[n] = nc.dram_tensor(n, shape, dt, kind="ExternalOutput").ap()

    with (
        tile.TileContext(nc) as tc,
        nc.allow_low_precision(reason="fp8 matmuls, f32 residual path"),
        tc.tile_pool(name="glob", bufs=1) as glob,
        tc.tile_pool(name="ps", bufs=1, space="PSUM") as ps,
    ):
        # ---- constants / persistent tiles ----
        vec = glob.tile([P, NVEC], F32, tag="vec")
        nc.sync.dma_start(vec[:], vec_d[:])
        ones2 = glob.tile([P, P], F32R, tag="ones2")
        nc.sync.dma_start(ones2[:], ones2_d[:])
        ones8 = glob.tile([P, 2, 16], F8, tag="ones8")
        nc.vector.memset(ones8[:], 1.0)
        h_t = glob.tile([P, KC, T], F32, tag="h")       # LN2 out (residual)
        hf8 = glob.tile([P, C2, 2, T], F8, tag="hf8")   # LN2 out fp8 (FFN1)
        hl8 = glob.tile([P, C2, 2, T], F8, tag="hl8")   # fp8 residual of h
        hs8 = glob.tile([P, C2, 2, T], F8, tag="hs8")   # h/64 for W1lo pass

        def ps_mm():
            return ps.tile([P, T], F32, tag="mm", bufs=2, name="psmm")

        def ps_sc():
            return ps.tile([P, 2, T], F32, tag="sc", bufs=2, name="pssc")

        def ps_av():
            return ps.tile([P, T], F32, tag="av", bufs=2, name="psav")

        # ============================ attention ============================
        def proj_v_units(pool, kv8, wv_t, on_act=False):
            """V projection units -> vt [P, i2, fold, head, 0:D] fp8."""
            vt = pool.tile([P, C2, 2, NH, VP], F8, tag="vt", bufs=2)
            nc.vector.memset(vt[:, :, :, :, D:D + 1], 1.0)

            def unit(i, n):
                pv = ps_mm()
                for c2 in range(C2):
                    nc.tensor.matmul(
                        pv[:, 0:NS], kv8[:, c2, :, i * P:(i + 1) * P],
                        wv_t[:, c2, :, n * NS:(n + 1) * NS],
                        start=(c2 == 0), stop=(c2 == C2 - 1), perf_mode=DR)
                dst = vt[:, i // 2, i % 2, 4 * n:4 * (n + 1), 0:D]
                src_ = pv[:, 0:NS].rearrange("p (h d) -> p h d", d=D)
                if on_act:
                    nc.scalar.copy(dst, src_)
                else:
                    nc.vector.tensor_copy(dst, src_)

            units = [(unit, i, n) for i in range(KC) for n in range(4)]
            return vt, units

        def proj_v(pool, kv8, wv_t, on_act=False):
            vt, units = proj_v_units(pool, kv8, wv_t, on_act)
            for u, i, n in units:
                u(i, n)
            return vt

        def proj_kf_units(pool, kv8, wk_t, kb, on_act):
            """K projection units, d-folded output kf[32b+p, a, g, s]."""
            kf = pool.tile([P, C2, 2, S], F8, tag="kf", bufs=2)

            def unit(a, g, n2):
                pk = ps_mm()
                for c2 in range(C2):
                    nc.tensor.matmul(
                        pk[:], wk_t[:, c2, a, g, :, :],
                        kv8[:, c2, :, n2 * T:(n2 + 1) * T],
                        start=(c2 == 0), stop=(c2 == C2 - 1), perf_mode=DR)
                col = kb + 2 * a + g
                dst = kf[:, a, g, n2 * T:(n2 + 1) * T]
                if on_act:
                    nc.scalar.activation(dst, pk[:], AF.Identity,
                                         bias=vec[:, col:col + 1])
                else:
                    nc.vector.tensor_scalar(dst, pk[:], vec[:, col:col + 1],
                                            None, op0=OP.add)

            units = [(unit, a, g, n2) for a in range(4) for g in range(2)
                     for n2 in range(2)]
            return kf, units

        def proj_kf(pool, kv8, wk_t, kb, on_act):
            kf, units = proj_kf_units(pool, kv8, wk_t, kb, on_act)
            for u, a, g, n2 in units:
                u(a, g, n2)
            return kf

        def proj_qf(pool, q8, wq_t, qb, on_act=True):
            qf = pool.tile([P, C2, 2, T], F8, tag="qf")
            for a in range(4):
                for g in range(2):
                    pq = ps_mm()
                    for c2 in range(C2):
                        nc.tensor.matmul(
                            pq[:], wq_t[:, c2, a, g, :, :], q8[:, c2, :, :],
                            start=(c2 == 0), stop=(c2 == C2 - 1), perf_mode=DR)
                    col = qb + 2 * a + g
                    if on_act:
                        nc.scalar.activation(qf[:, a, g, :], pq[:],
                                             AF.Identity,
                                             bias=vec[:, col:col + 1])
                    else:
                        nc.vector.tensor_scalar(qf[:, a, g, :], pq[:],
                                                vec[:, col:col + 1], None,
                                                op0=OP.add)
            return qf

        def scores_av(pool, kf, qf, vt, ab, hooks=None):
            """Per-head softmax(scores)@V -> at8 [P, k2, fold, T] fp8."""
            at8 = pool.tile([P, C2, 2, T], F8, tag="at8")
            hidx = 0
            for m in range(KC):
                for h2 in (1, 0):
                    if hooks and hidx in hooks:
                        hooks[hidx]()
                    hidx += 1
                    h = 2 * m + h2
                    a, b_ = h // 4, h % 4
                    psAv = ps_av()
                    for i2 in range(C2):
                        sc = ps_sc()
                        for j in (0, 1):
                            i = 2 * i2 + j
                            nc.tensor.matmul(
                                sc[:, j, :],
                                kf[32 * b_:32 * b_ + 32, a, :,
                                   i * P:(i + 1) * P],
                                qf[32 * b_:32 * b_ + 32, a, :, :],
                                start=True, stop=True, perf_mode=DR,
                                tile_position=(32 * b_, 0))
                        et = pool.tile([P, 2, T], F8, tag="et", bufs=4)
                        if masked:
                            for j in (0, 1):
                                col = ab + 2 * i2 + j
                                nc.scalar.activation(
                                    et[:, j, :], sc[:, j, :], AF.Exp,
                                    bias=vec[:, col:col + 1],
                                    scale=1.0 / (SCALE * SCALE))
                        else:
                            nc.scalar.activation(et[:], sc[:], AF.Exp,
                                                 scale=1.0 / (SCALE * SCALE))
                        nc.tensor.matmul(psAv[0:D + 1, :],
                                         vt[:, i2, :, h, 0:D + 1], et[:],
                                         start=(i2 == 0), stop=(i2 == C2 - 1),
                                         perf_mode=DR)
                    rden = pool.tile([1, T], F32R, tag="rden", bufs=2)
                    nc.vector.reciprocal(rden[:], psAv[D:D + 1, :])
                    psB = ps_av()
                    nc.tensor.matmul(psB[0:D, :], ones2[0:1, 0:D], rden[:],
                                     start=True, stop=True)
                    rb = pool.tile([D, T], F32, tag="rb", bufs=2)
                    nc.vector.tensor_copy(rb[:], psB[0:D, :])
                    if h2 == 0:
                        nc.vector.tensor_tensor(
                            at8[0:D, m // 2, m % 2, :], psAv[0:D, :],
                            rb[:], op=OP.mult)
                    else:
                        atmp = pool.tile([D, T], F8, tag="atmp", bufs=1)
                        nc.vector.tensor_tensor(atmp[:], psAv[0:D, :],
                                                rb[:], op=OP.mult)
                        nc.gpsimd.dma_start(at8[D:P, m // 2, m % 2, :],
                                            atmp[:])
            return at8

        def proj_o(pool, at8, wo_t, ob, resid, out_f8=None, out_f32=None,
                   ln_hook=None):
            """Out-proj + bias' + residual; optional per-chunk LN-sum hook."""
            for m in range(KC):
                po = ps_mm()
                for k2 in range(C2):
                    nc.tensor.matmul(po[:], wo_t[:, k2, :, m, :],
                                     at8[:, k2, :, :],
                                     start=(k2 == 0), stop=(k2 == C2 - 1),
                                     perf_mode=DR)
                dst = (out_f8[:, m // 2, m % 2, :] if out_f8 is not None
                       else out_f32[:, m, :])
                nc.vector.scalar_tensor_tensor(
                    dst, po[:], vec[:, ob + m:ob + m + 1], resid[:, m, :],
                    op0=OP.add, op1=OP.add)
                if ln_hook is not None:
                    ln_hook(m)

        # ======================== layernorm helpers ========================
        def ln_math(pool, psS, psQ):
            """[1,T] sums -> sc tile with rstd bcast [:,0,:], mean bcast
            [:,1,:]."""
            mean = pool.tile([1, T], F32, tag="lnv", bufs=3)
            nc.scalar.mul(mean[:], psS, 1.0 / H)
            ex2 = pool.tile([1, T], F32, tag="lnv", bufs=3)
            nc.scalar.mul(ex2[:], psQ, 1.0 / H)
            var = pool.tile([1, T], F32, tag="lnv", bufs=3)
            nc.vector.tensor_tensor(var[:], mean[:], mean[:], op=OP.mult)
            nc.vector.tensor_tensor(var[:], ex2[:], var[:], op=OP.subtract)
            lv = pool.tile([1, T], F32, tag="lnv", bufs=3)
            nc.scalar.activation(lv[:], var[:], AF.Ln,
                                 bias=vec[0:1, C_EPS:C_EPS + 1])
            rstd = pool.tile([1, T], F32R, tag="lnr", bufs=2)
            nc.scalar.activation(rstd[:], lv[:], AF.Exp, scale=-0.5)
            meanr = pool.tile([1, T], F32R, tag="lnr", bufs=2)
            nc.vector.tensor_copy(meanr[:], mean[:])
            scB = ps_sc()
            nc.tensor.matmul(scB[:, 0, :], ones2[0:1, :], rstd[:],
                             start=True, stop=True)
            nc.tensor.matmul(scB[:, 1, :], ones2[0:1, :], meanr[:],
                             start=True, stop=True)
            sbB = pool.tile([P, 2, T], F32, tag="sbB", bufs=1)
            nc.vector.tensor_copy(sbB[:], scB[:])
            return sbB

        def ln_stats_f8(pool, src8):
            """LN sums from an fp8-folded [P, C2, 2, T] tensor."""
            scS = ps_sc()
            for j in range(C2):
                nc.tensor.matmul(scS[0:1, 0, :], ones8[:, :, 0:1],
                                 src8[:, j, :, :], start=(j == 0),
                                 stop=(j == C2 - 1), perf_mode=DR,
                                 skip_group_check=True)
            for j in range(C2):
                sq8 = pool.tile([P, 2, T], F8, tag="sq8", bufs=2)
                nc.gpsimd.tensor_tensor(sq8[:], src8[:, j, :, :],
                                        src8[:, j, :, :], op=OP.mult)
                nc.tensor.matmul(scS[0:1, 1, :], ones8[:, :, 0:1], sq8[:],
                                 start=(j == 0), stop=(j == C2 - 1),
                                 perf_mode=DR, skip_group_check=True)
            return scS

        def ln_dst(pool, src_ap, sbB, m, dst_ap):
            """dst = g*(src - mean)*rstd + b for one [P, T] chunk."""
            e1 = nc.gpsimd if m % 2 == 0 else nc.vector
            e2 = nc.vector if m % 2 == 0 else nc.gpsimd
            t1 = pool.tile([P, T], F32, tag="t1", bufs=3)
            e1.tensor_tensor(t1[:], src_ap, sbB[:, 1, :], op=OP.subtract)
            if gbtriv:
                e2.tensor_tensor(dst_ap, t1[:], sbB[:, 0, :], op=OP.mult)
            else:
                e2.tensor_tensor(t1[:], t1[:], sbB[:, 0, :], op=OP.mult)
                e1.tensor_scalar(dst_ap, t1[:], vec[:, C_G + m:C_G + m + 1],
                                 vec[:, C_B + m:C_B + m + 1], op0=OP.mult,
                                 op1=OP.add)

        # =========================== main program ==========================
        with tc.tile_pool(name="attn", bufs=1) as pool:
            def load_w(tag, shape, dram):
                t = pool.tile(shape, F8, tag=tag, bufs=1, name=tag)
                nc.sync.dma_start(t[:], dram[:])
                return t

            # V-proj weights first (first consumer of xk8), then stream xk
            wv_s = load_w("wv", [P, C2, 2, H], w8["sv"])
            xk8 = pool.tile([P, C2, 2, S], F8, tag="xk8")
            xkr = xk_d.rearrange("(c p) t -> p c t", p=P)
            for blk in range(8):
                stg = pool.tile([P, KC, P], F32, tag="stg", bufs=2)
                nc.sync.dma_start(stg[:], xkr[:, :, blk * P:(blk + 1) * P])
                nc.gpsimd.tensor_copy(
                    xk8.rearrange("p c f (b t) -> p c f b t", t=P)
                    [:, :, :, blk, :],
                    stg.rearrange("p (c f) t -> p c f t", f=2)[:])
            # K weights + query slice after the hidden stream
            wk_s = pool.tile([P, C2, 4, 2, 2, P], F8, tag="wk", bufs=1,
                             name="wk")
            nc.sync.dma_start(wk_s[:, :, 0:2, :, :, :],
                              w8["skf"][:, :, 0:2, :, :, :])
            xq = pool.tile([P, KC, T], F32, tag="xq")
            nc.sync.dma_start(xq[:], xq_d.rearrange("(c p) t -> p c t", p=P))
            xq8 = pool.tile([P, C2, 2, T], F8, tag="xq8")
            for c in range(KC):
                nc.gpsimd.tensor_copy(xq8[:, c // 2, c % 2, :], xq[:, c, :])
            wq_s = pool.tile([P, C2, 4, 2, 2, P], F8, tag="wq", bufs=1,
                             name="wq")
            nc.sync.dma_start(wq_s[:, :, 0:2, :, :, :],
                              w8["sqf"][:, :, 0:2, :, :, :])
            nc.sync.dma_start(wk_s[:, :, 2:4, :, :, :],
                              w8["skf"][:, :, 2:4, :, :, :])
            nc.sync.dma_start(wq_s[:, :, 2:4, :, :, :],
                              w8["sqf"][:, :, 2:4, :, :, :])

            # ---- cross hidden staging + fold (early; overlaps self attn) ----
            xc8 = pool.tile([P, C2, 2, S], F8, tag="xc8")
            xcr = xc_d.rearrange("(c p) t -> p c t", p=P)
            for j in range(KC):
                stg = pool.tile([P, S], F32, tag="stg", bufs=2)
                nc.sync.dma_start(stg[:], xcr[:, j, :])
                nc.gpsimd.tensor_copy(xc8[:, j // 2, j % 2, :], stg[:])
            wo_s = load_w("wo", [P, C2, 2, KC, P], w8["so"])
            # cross weights stream while self attention computes
            wv_c = load_w("wv", [P, C2, 2, H], w8["cv"])
            wk_c = load_w("wk", [P, C2, 4, 2, 2, P], w8["ckf"])
            wq_c = load_w("wq", [P, C2, 4, 2, 2, P], w8["cqf"])
            wo_c = load_w("wo", [P, C2, 2, KC, P], w8["co"])
            w1t0 = glob.tile([P, 2, C2, 2, 4, P], F8, tag="w1t0")
            nc.sync.dma_start(w1t0[:], w18_d[:, 0, :, :, :, 0:4, :])

            vt = proj_v(pool, xk8, wv_s, on_act=False)
            kf = proj_kf(pool, xk8, wk_s, C_SBK, on_act=False)
            qf = proj_qf(pool, xq8, wq_s, C_SBQ, on_act=False)
            vt_c, v_units = proj_v_units(pool, xc8, wv_c)
            kf_c, k_units = proj_kf_units(pool, xc8, wk_c, C_CBK,
                                          on_act=False)
            work = ([("v",) + u[1:] for u in v_units]
                    + [("k",) + u[1:] for u in k_units])
            v_unit, k_unit = v_units[0][0], k_units[0][0]
            hooks = {}
            for hi in range(2, 14):
                lo = (hi - 2) * 4
                chunk = work[lo:lo + 4]

                def _mk(chunk):
                    def _h():
                        for w in chunk:
                            if w[0] == "v":
                                v_unit(w[1], w[2])
                            else:
                                k_unit(w[1], w[2], w[3])
                    return _h

                hooks[hi] = _mk(chunk)
            at8 = scores_av(pool, kf, qf, vt, C_SAB, hooks=hooks)
            if dbg:
                nc.sync.dma_start(dbg_d["d_kf"][:], kf[:])
                nc.sync.dma_start(dbg_d["d_qf"][:], qf[:])
                nc.sync.dma_start(dbg_d["d_vt"][:], vt[:])
                nc.sync.dma_start(dbg_d["d_at"][:], at8[:])
            sa8 = pool.tile([P, C2, 2, T], F8, tag="sa8")
            acc1 = {}

            def ln1_hook(m):
                if m % 2 == 0:
                    return
                j = m // 2
                if "scS" not in acc1:
                    acc1["scS"] = ps_sc()
                scS = acc1["scS"]
                nc.tensor.matmul(scS[0:1, 0, :], ones8[:, :, 0:1],
                                 sa8[:, j, :, :], start=(j == 0),
                                 stop=(j == C2 - 1), perf_mode=DR,
                                 skip_group_check=True)
                sq8 = pool.tile([P, 2, T], F8, tag="sq8", bufs=2)
                nc.gpsimd.tensor_tensor(sq8[:], sa8[:, j, :, :],
                                        sa8[:, j, :, :], op=OP.mult)
                nc.tensor.matmul(scS[0:1, 1, :], ones8[:, :, 0:1], sq8[:],
                                 start=(j == 0), stop=(j == C2 - 1),
                                 perf_mode=DR, skip_group_check=True)

            proj_o(pool, at8, wo_s, C_SBO, xq, out_f8=sa8, ln_hook=ln1_hook)
            if dbg:
                nc.sync.dma_start(dbg_d["d_sa"][:], sa8[:])

            # ---- LN1 stats + fused cross-Q ----
            # q_c = rstd * (sa@Wq' - mean*colsum(Wq')) + bias'; the Wq'
            # matmuls consume sa8 directly, overlapping the LN1 math.
            scS1 = acc1["scS"]
            qf_c = pool.tile([P, C2, 2, T], F8, tag="qf")
            scB1 = None
            for a in range(4):
                for g in range(2):
                    i_ = 2 * a + g
                    pq = ps_mm()
                    for c2 in range(C2):
                        nc.tensor.matmul(
                            pq[:], wq_c[:, c2, a, g, :, :], sa8[:, c2, :, :],
                            start=(c2 == 0), stop=(c2 == C2 - 1), perf_mode=DR)
                    if scB1 is None:
                        scB1 = ln_math(pool, scS1[0:1, 0, :],
                                       scS1[0:1, 1, :])
                    t1 = pool.tile([P, T], F32, tag="t1", bufs=3)
                    nc.vector.scalar_tensor_tensor(
                        t1[:], scB1[:, 1, :],
                        vec[:, C_CQS + i_:C_CQS + i_ + 1], pq[:],
                        op0=OP.mult, op1=OP.add)
                    nc.vector.tensor_tensor(t1[:], t1[:], scB1[:, 0, :],
                                            op=OP.mult)
                    nc.scalar.activation(qf_c[:, a, g, :], t1[:], AF.Identity,
                                         bias=vec[:, C_CBQ + i_:
                                                  C_CBQ + i_ + 1])
            at8_c = scores_av(pool, kf_c, qf_c, vt_c, C_CAB)
            ca = pool.tile([P, KC, T], F32R, tag="ca")
            acc2 = {}

            def ln2_hook(m):
                # ride LN2 sums on ca chunks as they complete
                if "scS" not in acc2:
                    acc2["scS"] = ps_sc()
                scS = acc2["scS"]
                nc.tensor.matmul(scS[0:1, 0, :], ones2[:, 0:1], ca[:, m, :],
                                 start=(m == 0), stop=(m == KC - 1),
                                 skip_group_check=True)
                sq = pool.tile([P, T], F32R, tag="sq", bufs=2)
                nc.vector.tensor_tensor(sq[:], ca.bitcast(F32)[:, m, :],
                                        ca.bitcast(F32)[:, m, :], op=OP.mult)
                nc.tensor.matmul(scS[0:1, 1, :], ones2[:, 0:1], sq[:],
                                 start=(m == 0), stop=(m == KC - 1),
                                 skip_group_check=True)

            proj_o(pool, at8_c, wo_c, C_CBO, xq, out_f32=ca,
                   ln_hook=ln2_hook)

            # ---- LN2 (on ca) -> h (f32) + hf8 ----
            scS2 = acc2["scS"]
            scB2 = ln_math(pool, scS2[0:1, 0, :], scS2[0:1, 1, :])
            for m in range(KC):
                ln_dst(pool, ca.bitcast(F32)[:, m, :], scB2, m, h_t[:, m, :])
                nc.scalar.copy(hf8[:, m // 2, m % 2, :], h_t[:, m, :])
                nc.gpsimd.tensor_tensor(hl8[:, m // 2, m % 2, :],
                                        h_t[:, m, :],
                                        hf8[:, m // 2, m % 2, :],
                                        op=OP.subtract)
                nc.scalar.mul(hs8[:, m // 2, m % 2, :], h_t[:, m, :],
                              1.0 / 64.0)
            if dbg:
                nc.sync.dma_start(
                    dbg_d["d_h"].rearrange("(c p) t -> p c t", p=P), h_t[:])

        # ================= FFN (fp8 DoubleRow) + final LN ==================
        with tc.tile_pool(name="ffn", bufs=1) as pool:
            ut8 = pool.tile([P, 16, 2, T], F8, tag="ut8")
            us8 = pool.tile([P, 16, 2, T], F8, tag="us8")
            for m0 in range(0, 32, 4):
                if m0 == 0:
                    w1t = w1t0
                else:
                    w1t = pool.tile([P, 2, C2, 2, 4, P], F8, tag="w1t",
                                    bufs=2)
                    q = m0 // 4
                    nc.sync.dma_start(
                        w1t[:], w18_d[:, q // 2, :, :, :,
                                      (q % 2) * 4:(q % 2) * 4 + 4, :])
                for m in range(m0, m0 + 4):
                    pu = ps_mm()
                    passes = [(0, hf8), (0, hl8), (1, hs8)]
                    for pi, (lo, hsrc) in enumerate(passes):
                        for c2 in range(C2):
                            nc.tensor.matmul(
                                pu[:], w1t[:, lo, c2, :, m - m0, :],
                                hsrc[:, c2, :, :],
                                start=(pi == 0 and c2 == 0),
                                stop=(pi == 2 and c2 == C2 - 1),
                                perf_mode=DR)
                    nc.vector.tensor_scalar(ut8[:, m // 2, m % 2, :], pu[:],
                                      vec[:, C_B1 + m:C_B1 + m + 1], 0.0,
                                      op0=OP.add, op1=OP.max)
                    nc.gpsimd.tensor_scalar(us8[:, m // 2, m % 2, :],
                                            ut8[:, m // 2, m % 2, :],
                                            1.0 / 64.0, None, op0=OP.mult)
            if dbg:
                nc.sync.dma_start(dbg_d["d_u"][:], ut8[:])

            ff = pool.tile([P, KC, T], F32R, tag="ff")
            acc3 = {}
            for m in range(KC):
                w2t = pool.tile([P, 2, 16, 2, P], F8, tag="w2t", bufs=3)
                nc.sync.dma_start(w2t[:], w28_d[:, m, :, :, :, :])
                pf = ps_mm()
                for lo, usrc in ((0, ut8), (1, us8)):
                    for k2 in range(16):
                        nc.tensor.matmul(pf[:], w2t[:, lo, k2, :, :],
                                         usrc[:, k2, :, :],
                                         start=(lo == 0 and k2 == 0),
                                         stop=(lo == 1 and k2 == 15),
                                         perf_mode=DR)
                nc.vector.scalar_tensor_tensor(
                    ff[:, m, :], pf[:], vec[:, C_B2 + m:C_B2 + m + 1],
                    h_t[:, m, :], op0=OP.add, op1=OP.add)
                if "scS" not in acc3:
                    acc3["scS"] = ps_sc()
                scS = acc3["scS"]
                nc.tensor.matmul(scS[0:1, 0, :], ones2[:, 0:1], ff[:, m, :],
                                 start=(m == 0), stop=(m == KC - 1),
                                 skip_group_check=True)
                sq = pool.tile([P, T], F32R, tag="sq", bufs=2)
                nc.gpsimd.tensor_tensor(sq[:], ff.bitcast(F32)[:, m, :],
                                        ff.bitcast(F32)[:, m, :], op=OP.mult)
                nc.tensor.matmul(scS[0:1, 1, :], ones2[:, 0:1], sq[:],
                                 start=(m == 0), stop=(m == KC - 1),
                                 skip_group_check=True)

            scS3 = acc3["scS"]
            scB3 = ln_math(pool, scS3[0:1, 0, :], scS3[0:1, 1, :])
            for m in range(KC):
                ob = pool.tile([P, T], F32, tag="ob", bufs=3)
                ln_dst(pool, ff.bitcast(F32)[:, m, :], scB3, m, ob[:])
                nc.sync.dma_start(out_d[m * P:(m + 1) * P, :], ob[:])

    _legalize_waits(nc)
    return nc
